# revision 3
# baseline (speedup 1.0000x reference)
"""Class-conditional label-smoothing cross-entropy loss on 8 Trainium2 cores.

Reference math (C=1000 classes, B=65536 samples, smoothing s=0.1):
    A = softmax(class_avg, axis=-1)                         # [C, C]
    S[t, j] = s * (1 - A[t, j]) / (1 - A[t, t])  (j != t);  S[t, t] = 1 - s
    R[t]    = sum_j S[t, j]
    loss_i  = lse_i * R[t_i] - S[t_i] . x_i,   lse_i = log(sum_j exp(x_ij))
    out     = mean_i loss_i

Data-parallel: x and target are sharded along batch across the 8 cores,
class_avg is replicated. Each core:
  1. builds the smoothing table in its DRAM once:
     tab[t] = [S[t, :] as bf16 (1000) | R[t] as f32 bit-packed in 2 bf16
     slots | zero pad to 1024]   (bf16 halves the per-sample gather traffic;
     since E[x]=0 the S quantization does not bias the mean loss, and R stays
     exact f32 via the bit-pack)
  2. processes 64 tiles of 128 samples (sample p*64+j -> tile j, partition p):
     x tile DMA, indirect-DMA row gather of tab by target, ACT exp with
     accumulate -> sumexp, one fused DVE multiply with accumulate -> dot
  3. tail: lse = ln(sumexp), loss = R*lse - dot, one [128, 64] store.
Host sums the 8 partial grids in f64 and divides by B.
"""

import numpy as np

import concourse.bass as bass
import concourse.tile as tile
from concourse import bacc, mybir
from concourse.bass_utils import run_bass_kernel_spmd

B = 65536
C = 1000
NCORES = 8
BLOC = B // NCORES          # 8192 samples per core
P = 128
NT = BLOC // P              # 64 sample tiles per core
TABW = 1024                 # table row: 1000 bf16 S + f32 R (2 slots) + pad
SM = 0.1

_CACHE = {}


def build_program():
    f32 = mybir.dt.float32
    bf16 = mybir.dt.bfloat16
    i32 = mybir.dt.int32
    Alu = mybir.AluOpType
    Act = mybir.ActivationFunctionType

    nc = bacc.Bacc("TRN2", target_bir_lowering=False, debug=False)
    x_ap = nc.dram_tensor("x", [BLOC, C], f32, kind="ExternalInput").ap()
    ca_ap = nc.dram_tensor("ca", [C, C], f32, kind="ExternalInput").ap()
    tg_ap = nc.dram_tensor("tg", [BLOC], i32, kind="ExternalInput").ap()
    out_ap = nc.dram_tensor("out", [P, NT], f32, kind="ExternalOutput").ap()
    tab_ap = nc.dram_tensor("tab", [C, TABW], bf16).ap()

    with tile.TileContext(nc) as tc:
        with (
            tc.tile_pool(name="tabp", bufs=2) as tabp,
            tc.tile_pool(name="small", bufs=2) as small,
            tc.tile_pool(name="xs", bufs=3) as xs,
            tc.tile_pool(name="gs", bufs=3) as gs,
            tc.tile_pool(name="scr", bufs=2) as scr,
            tc.tile_pool(name="cols", bufs=1) as cols,
        ):
            # target indices: idx[p, j] = tg[p*NT + j]
            idx = cols.tile([P, NT], i32)
            nc.sync.dma_start(idx[:], tg_ap.rearrange("(p c) -> p c", c=NT))

            # ---- smoothing table -------------------------------------------
            for k in range((C + P - 1) // P):
                r0 = k * P
                pr = min(r0 + P, C) - r0
                cat = tabp.tile([P, C], f32, tag="cat")
                nc.sync.dma_start(cat[:pr], ca_ap[r0 : r0 + pr, :])
                e = tabp.tile([P, C], f32, tag="e")
                sume = small.tile([P, 1], f32, tag="sume")
                nc.scalar.activation(e[:pr], cat[:pr], Act.Exp, accum_out=sume[:pr])
                # diagonal e[t, t] via affine mask + row reduce
                msk = tabp.tile([P, C], f32, tag="msk")
                nc.gpsimd.affine_select(
                    out=msk[:pr], in_=e[:pr], compare_op=Alu.is_equal, fill=0.0,
                    base=-r0, channel_multiplier=-1, pattern=[[1, C]],
                )
                ett = small.tile([P, 1], f32, tag="ett")
                nc.vector.tensor_reduce(
                    out=ett[:pr], in_=msk[:pr], axis=mybir.AxisListType.X, op=Alu.add
                )
                den = small.tile([P, 1], f32, tag="den")
                nc.vector.tensor_tensor(
                    out=den[:pr], in0=sume[:pr], in1=ett[:pr], op=Alu.subtract
                )
                rec = small.tile([P, 1], f32, tag="rec")
                nc.vector.reciprocal(rec[:pr], den[:pr])
                negw = small.tile([P, 1], f32, tag="negw")
                nc.vector.tensor_scalar_mul(negw[:pr], rec[:pr], -SM)
                # S_pre[t, j] = (e - sume) * (-s / den); its diagonal equals s,
                # and sum_j S_pre = R - (1 - 2s)
                spre = tabp.tile([P, C], f32, tag="spre")
                rpre = small.tile([P, 1], f32, tag="rpre")
                nc.vector.scalar_tensor_tensor(
                    out=spre[:pr], in0=e[:pr], scalar=sume[:pr],
                    in1=negw[:pr].to_broadcast([pr, C]),
                    op0=Alu.subtract, op1=Alu.mult, accum_out=rpre[:pr],
                )
                sb = tabp.tile([P, TABW], bf16, tag="sb")
                nc.gpsimd.affine_select(
                    out=sb[:pr, 0:C], in_=spre[:pr], compare_op=Alu.not_equal,
                    fill=1.0 - SM, base=-r0, channel_multiplier=-1, pattern=[[1, C]],
                )
                # R as a hi/lo bf16 pair (R = hi + lo, error ~2^-18 relative)
                rt = small.tile([P, 1], f32, tag="rt")
                nc.vector.tensor_scalar_add(rt[:pr], rpre[:pr], 1.0 - 2 * SM)
                nc.vector.tensor_copy(out=sb[:pr, C : C + 1], in_=rt[:pr])
                nc.vector.tensor_tensor(
                    out=sb[:pr, C + 1 : C + 2], in0=rt[:pr],
                    in1=sb[:pr, C : C + 1], op=Alu.subtract,
                )
                nc.vector.memset(sb[:pr, C + 2 : TABW], 0.0)
                nc.sync.dma_start(tab_ap[r0 : r0 + pr, :], sb[:pr])

            # ---- main loop -------------------------------------------------
            x_r = x_ap.rearrange("(p c) d -> p c d", c=NT)
            se_cols = cols.tile([P, NT], f32)
            dot_cols = cols.tile([P, NT], f32)
            r_cols = cols.tile([P, NT], f32)
            for j in range(NT):
                xt = xs.tile([P, C], f32)
                nc.sync.dma_start(xt[:], x_r[:, j, :])
                gt = gs.tile([P, TABW], bf16)
                nc.gpsimd.indirect_dma_start(
                    out=gt[:], out_offset=None, in_=tab_ap[:],
                    in_offset=bass.IndirectOffsetOnAxis(ap=idx[:, j : j + 1], axis=0),
                )
                es = scr.tile([P, C], bf16, tag="es")
                nc.scalar.activation(
                    es[:], xt[:], Act.Exp, accum_out=se_cols[:, j : j + 1]
                )
                ps = scr.tile([P, C], f32, tag="ps")
                nc.vector.scalar_tensor_tensor(
                    out=ps[:], in0=xt[:], scalar=1.0, in1=gt[:, 0:C],
                    op0=Alu.mult, op1=Alu.mult, accum_out=dot_cols[:, j : j + 1],
                )
                nc.vector.tensor_tensor(
                    out=r_cols[:, j : j + 1], in0=gt[:, C : C + 1],
                    in1=gt[:, C + 1 : C + 2], op=Alu.add,
                )

            # ---- tail ------------------------------------------------------
            lse = cols.tile([P, NT], f32)
            nc.scalar.activation(lse[:], se_cols[:], Act.Ln)
            t1 = cols.tile([P, NT], f32)
            nc.vector.tensor_mul(t1[:], r_cols[:], lse[:])
            loss = cols.tile([P, NT], f32)
            nc.vector.tensor_tensor(
                out=loss[:], in0=t1[:], in1=dot_cols[:], op=Alu.subtract
            )
            nc.sync.dma_start(out_ap[:], loss[:])

    nc.compile()
    nc.finalize()
    return nc


def get_program():
    if "nc" not in _CACHE:
        _CACHE["nc"] = build_program()
    return _CACHE["nc"]


def make_in_maps(x, class_avg, target):
    x = np.ascontiguousarray(np.asarray(x, dtype=np.float32))
    ca = np.ascontiguousarray(np.asarray(class_avg, dtype=np.float32))
    tg = np.ascontiguousarray(np.asarray(target).astype(np.int32))
    assert x.shape == (B, C) and ca.shape == (C, C) and tg.shape == (B,)
    return [
        {"x": x[c * BLOC : (c + 1) * BLOC], "ca": ca, "tg": tg[c * BLOC : (c + 1) * BLOC]}
        for c in range(NCORES)
    ]


def reduce_outputs(results):
    tot = 0.0
    for c in range(NCORES):
        tot += results[c]["out"].astype(np.float64).sum()
    return np.array(tot / B, dtype=np.float32)


def kernel(x, class_avg, target):
    nc = get_program()
    in_maps = make_in_maps(x, class_avg, target)
    res = run_bass_kernel_spmd(nc, in_maps, list(range(NCORES)))
    return reduce_outputs(res.results)


# revision 5
# speedup vs baseline: 40.0290x; 40.0290x over previous
"""Class-conditional label-smoothing cross-entropy loss on 8 Trainium2 cores.

Reference math (C=1000 classes, B=65536 samples, smoothing s=0.1):
    A = softmax(class_avg, axis=-1)                         # [C, C]
    S[t, j] = s * (1 - A[t, j]) / (1 - A[t, t])  (j != t);  S[t, t] = 1 - s
    R[t]    = sum_j S[t, j]
    loss_i  = lse_i * R[t_i] - S[t_i] . x_i,   lse_i = log(sum_j exp(x_ij))
    out     = mean_i loss_i

Data-parallel: x and target are sharded along batch across the 8 cores,
class_avg is replicated. Each core:
  1. builds the smoothing table in its DRAM once:
     tab[t] = [S[t, :] as bf16 (1000) | R[t] as f32 bit-packed in 2 bf16
     slots | zero pad to 1024]   (bf16 halves the per-sample gather traffic;
     since E[x]=0 the S quantization does not bias the mean loss, and R stays
     exact f32 via the bit-pack)
  2. processes 64 tiles of 128 samples (sample p*64+j -> tile j, partition p):
     x tile DMA, indirect-DMA row gather of tab by target, ACT exp with
     accumulate -> sumexp, one fused DVE multiply with accumulate -> dot
  3. tail: lse = ln(sumexp), loss = R*lse - dot, one [128, 64] store.
Host sums the 8 partial grids in f64 and divides by B.
"""

import numpy as np

import concourse.bass as bass
import concourse.tile as tile
from concourse import bacc, mybir
from concourse.bass_utils import run_bass_kernel_spmd

B = 65536
C = 1000
NCORES = 8
BLOC = B // NCORES          # 8192 samples per core
P = 128
NT = BLOC // P              # 64 sample tiles per core
TABW = 1024                 # table row: 1000 bf16 S + f32 R (2 slots) + pad
SM = 0.1

_CACHE = {}


def build_program(reps=1):
    # reps>1 repeats the main loop body (same data) for slope-timing in
    # test.py: device time scales with reps, dispatch overhead does not.
    f32 = mybir.dt.float32
    bf16 = mybir.dt.bfloat16
    i32 = mybir.dt.int32
    Alu = mybir.AluOpType
    Act = mybir.ActivationFunctionType

    nc = bacc.Bacc("TRN2", target_bir_lowering=False, debug=False)
    x_ap = nc.dram_tensor("x", [BLOC, C], f32, kind="ExternalInput").ap()
    ca_ap = nc.dram_tensor("ca", [C, C], f32, kind="ExternalInput").ap()
    tg_ap = nc.dram_tensor("tg", [BLOC], i32, kind="ExternalInput").ap()
    out_ap = nc.dram_tensor("out", [P, NT], f32, kind="ExternalOutput").ap()
    tab_ap = nc.dram_tensor("tab", [C, TABW], bf16).ap()

    with tile.TileContext(nc) as tc:
        with (
            tc.tile_pool(name="tabp", bufs=2) as tabp,
            tc.tile_pool(name="small", bufs=2) as small,
            tc.tile_pool(name="xs", bufs=3) as xs,
            tc.tile_pool(name="gs", bufs=3) as gs,
            tc.tile_pool(name="scr", bufs=2) as scr,
            tc.tile_pool(name="cols", bufs=1) as cols,
        ):
            # target indices: idx[p, j] = tg[p*NT + j]
            idx = cols.tile([P, NT], i32)
            nc.sync.dma_start(idx[:], tg_ap.rearrange("(p c) -> p c", c=NT))

            # ---- smoothing table -------------------------------------------
            for k in range((C + P - 1) // P):
                r0 = k * P
                pr = min(r0 + P, C) - r0
                cat = tabp.tile([P, C], f32, tag="cat")
                nc.sync.dma_start(cat[:pr], ca_ap[r0 : r0 + pr, :])
                e = tabp.tile([P, C], f32, tag="e")
                sume = small.tile([P, 1], f32, tag="sume")
                nc.scalar.activation(e[:pr], cat[:pr], Act.Exp, accum_out=sume[:pr])
                # diagonal e[t, t] via affine mask + row reduce
                msk = tabp.tile([P, C], f32, tag="msk")
                nc.gpsimd.affine_select(
                    out=msk[:pr], in_=e[:pr], compare_op=Alu.is_equal, fill=0.0,
                    base=-r0, channel_multiplier=-1, pattern=[[1, C]],
                )
                ett = small.tile([P, 1], f32, tag="ett")
                nc.vector.tensor_reduce(
                    out=ett[:pr], in_=msk[:pr], axis=mybir.AxisListType.X, op=Alu.add
                )
                den = small.tile([P, 1], f32, tag="den")
                nc.vector.tensor_tensor(
                    out=den[:pr], in0=sume[:pr], in1=ett[:pr], op=Alu.subtract
                )
                rec = small.tile([P, 1], f32, tag="rec")
                nc.vector.reciprocal(rec[:pr], den[:pr])
                negw = small.tile([P, 1], f32, tag="negw")
                nc.vector.tensor_scalar_mul(negw[:pr], rec[:pr], -SM)
                # S_pre[t, j] = (e - sume) * (-s / den); its diagonal equals s,
                # and sum_j S_pre = R - (1 - 2s)
                spre = tabp.tile([P, C], f32, tag="spre")
                rpre = small.tile([P, 1], f32, tag="rpre")
                nc.vector.scalar_tensor_tensor(
                    out=spre[:pr], in0=e[:pr], scalar=sume[:pr],
                    in1=negw[:pr].to_broadcast([pr, C]),
                    op0=Alu.subtract, op1=Alu.mult, accum_out=rpre[:pr],
                )
                sb = tabp.tile([P, TABW], bf16, tag="sb")
                nc.gpsimd.affine_select(
                    out=sb[:pr, 0:C], in_=spre[:pr], compare_op=Alu.not_equal,
                    fill=1.0 - SM, base=-r0, channel_multiplier=-1, pattern=[[1, C]],
                )
                # R as a hi/lo bf16 pair (R = hi + lo, error ~2^-18 relative)
                rt = small.tile([P, 1], f32, tag="rt")
                nc.vector.tensor_scalar_add(rt[:pr], rpre[:pr], 1.0 - 2 * SM)
                nc.vector.tensor_copy(out=sb[:pr, C : C + 1], in_=rt[:pr])
                nc.vector.tensor_tensor(
                    out=sb[:pr, C + 1 : C + 2], in0=rt[:pr],
                    in1=sb[:pr, C : C + 1], op=Alu.subtract,
                )
                nc.vector.memset(sb[:pr, C + 2 : TABW], 0.0)
                nc.sync.dma_start(tab_ap[r0 : r0 + pr, :], sb[:pr])

            # ---- main loop -------------------------------------------------
            x_r = x_ap.rearrange("(p c) d -> p c d", c=NT)
            se_cols = cols.tile([P, NT], f32)
            dot_cols = cols.tile([P, NT], f32)
            r_cols = cols.tile([P, NT], f32)
            for j in range(NT * reps):
                j = j % NT
                xt = xs.tile([P, C], f32)
                nc.sync.dma_start(xt[:], x_r[:, j, :])
                gt = gs.tile([P, TABW], bf16)
                nc.gpsimd.indirect_dma_start(
                    out=gt[:], out_offset=None, in_=tab_ap[:],
                    in_offset=bass.IndirectOffsetOnAxis(ap=idx[:, j : j + 1], axis=0),
                )
                es = scr.tile([P, C], bf16, tag="es")
                nc.scalar.activation(
                    es[:], xt[:], Act.Exp, accum_out=se_cols[:, j : j + 1]
                )
                ps = scr.tile([P, C], f32, tag="ps")
                nc.vector.scalar_tensor_tensor(
                    out=ps[:], in0=xt[:], scalar=1.0, in1=gt[:, 0:C],
                    op0=Alu.mult, op1=Alu.mult, accum_out=dot_cols[:, j : j + 1],
                )
                nc.vector.tensor_tensor(
                    out=r_cols[:, j : j + 1], in0=gt[:, C : C + 1],
                    in1=gt[:, C + 1 : C + 2], op=Alu.add,
                )

            # ---- tail ------------------------------------------------------
            lse = cols.tile([P, NT], f32)
            nc.scalar.activation(lse[:], se_cols[:], Act.Ln)
            t1 = cols.tile([P, NT], f32)
            nc.vector.tensor_mul(t1[:], r_cols[:], lse[:])
            loss = cols.tile([P, NT], f32)
            nc.vector.tensor_tensor(
                out=loss[:], in0=t1[:], in1=dot_cols[:], op=Alu.subtract
            )
            nc.sync.dma_start(out_ap[:], loss[:])

    nc.compile()
    nc.finalize()
    return nc


def get_program():
    if "nc" not in _CACHE:
        _CACHE["nc"] = build_program()
    return _CACHE["nc"]


def make_in_maps(x, class_avg, target):
    x = np.ascontiguousarray(np.asarray(x, dtype=np.float32))
    ca = np.ascontiguousarray(np.asarray(class_avg, dtype=np.float32))
    tg = np.ascontiguousarray(np.asarray(target).astype(np.int32))
    assert x.shape == (B, C) and ca.shape == (C, C) and tg.shape == (B,)
    return [
        {"x": x[c * BLOC : (c + 1) * BLOC], "ca": ca, "tg": tg[c * BLOC : (c + 1) * BLOC]}
        for c in range(NCORES)
    ]


def reduce_outputs(results):
    tot = 0.0
    for c in range(NCORES):
        tot += results[c]["out"].astype(np.float64).sum()
    return np.array(tot / B, dtype=np.float32)


def kernel(x, class_avg, target):
    nc = get_program()
    in_maps = make_in_maps(x, class_avg, target)
    res = run_bass_kernel_spmd(nc, in_maps, list(range(NCORES)))
    return reduce_outputs(res.results)
